# revision 29
# baseline (speedup 1.0000x reference)
"""ConvDecoder Bass kernel for Trainium2, SPMD over 8 NeuronCores.

Math (per batch element b, one per core):
    r_conv = Conv1d(r, conv_w, SAME) + conv_b            # (C, N_IN)
    d[n,m] = (xc[n] - xt[m])^2                           # (N_IN, N_OUT)
    E_c    = exp(-a_c * d),  a_c = 0.5 / exp(sigma_c)^2
    z[m,c] = sum_n r_conv[c,n] * E_c[n,m]
    out    = z @ lin_w.T + lin_b                         # (N_OUT, OUT_C)

Per-core structure (v7):
  - The pointwise linear is FOLDED INTO THE CONV WEIGHTS on host:
    W2_g = w_aug[:, c_g] @ lin_w.T[c_g] (+ lin_b on the im2col ones
    row), one block per length-scale group g.  The conv matmul then
    yields R2[n, o] = sum_c r_conv[n, c] lin_w[o, c] directly, and
    the z matmul produces y^T = sum_n R2[n,:]^T E[n,:] -- the output
    itself in (OUT_C, m) layout.  No on-device linear stage at all;
    the host transposes (free) when assembling.
  - im2col stack built on HOST in sorted-xc order; W2 rides in its
    trailing columns (ONE input DMA for all weights/data except uv).
  - d^2 on the PE as a rank-7 bf16 matmul (exact hi/lo splits), one
    row-tile position per n-tile so the matmuls overlap; ACT exp
    reads PSUM, writes bf16 E to SBUF.  ACT is the serial bottleneck.
  - BANDING: xc/xt sorted on host => far (n-tile, m-half) blocks are
    skipped (mask from the actual data, union over batches, baked
    into the compiled kernel).  With the usual 6/8 mask each dsq
    n-tile gets a DEDICATED PSUM slot (no rotation, d^2 never waits
    on ACT); the conv shares n-tile 0's bank after its exp drains.
  - y^T accumulates in two PSUM banks (m-half 0 -> partitions 0:32
    col group 0, m-half 1 -> 32:64 col group 1, concurrent), drained
    in parallel by DVE/ACT, stored by two contiguous 64KB DMAs in
    sorted order; host inverse-gathers rows.
"""

import numpy as np
import ml_dtypes

import concourse.bass as bass
import concourse.mybir as mybir
from concourse.tile import TileContext
from concourse.bass_utils import run_bass_kernel_spmd

F32 = mybir.dt.float32
BF16 = mybir.dt.bfloat16
BF = ml_dtypes.bfloat16

B, N_IN, N_OUT, C, OUT_C, KW = 8, 512, 1024, 16, 32, 5
N_CORES = 8
NT = N_IN // 128   # n tiles (4)
MH = N_OUT // 512  # m halves (2)
ROWS = C * KW + 1  # im2col rows (81)
BAND_T = 16.0      # skip a block when a * gap^2 > BAND_T (exp < 1.2e-7)


# --- walrus workaround -----------------------------------------------------
# This container's walrus accepts at most ONE semaphore wait per TPB
# instruction, but Tile's scheduler attaches several (joins + tail drain).
# Hoist all but the last wait of each instruction onto fresh wait-only
# EventSemaphore instructions inserted right before it on the same engine.
_ws_ctr = [0]


def _split_multi_waits(nc):
    for fn in nc.m.functions:
        for blk in fn.blocks:
            insts = blk.instructions
            if not any(
                ins.sync_info and len(ins.sync_info.on_wait) > 1 for ins in insts
            ):
                continue
            out = []
            for ins in insts:
                si = ins.sync_info
                waits = list(si.on_wait) if si else []
                if len(waits) > 1:
                    for w in waits[:-1]:
                        _ws_ctr[0] += 1
                        ev = mybir.InstEventSemaphore(
                            name=f"waitsplit_{_ws_ctr[0]}", ins=[], outs=[]
                        )
                        ev.engine = ins.engine
                        ev.sync_info = mybir.SyncInfo(on_wait=[w], on_update=[])
                        nc.register_instruction(ev)
                        out.append(ev)
                    ins.sync_info = mybir.SyncInfo(
                        on_wait=[waits[-1]], on_update=list(si.on_update)
                    )
                out.append(ins)
            insts[:] = out


# --- kernel build ----------------------------------------------------------
def _build(key):
    """key: (n_groups, a_tuple, windows) - a_tuple the per-group exp
    scales, windows the per-n-tile active sorted-m column range."""
    n_groups, a_tuple, windows = key
    G = n_groups
    nc = bass.Bass()
    # stack carries the im2col block (cols 0:512) and the folded
    # conv+linear weights W2 per group (cols 512:512+32G)
    stack_d = nc.dram_tensor("stack", [ROWS, N_IN + G * OUT_C], BF16,
                             kind="ExternalInput")
    uv_d = nc.dram_tensor("uv", [8, N_IN + N_OUT], BF16, kind="ExternalInput")
    # output is y^T in sorted-m order; host transposes + inverse-gathers
    y_d = nc.dram_tensor("y", [OUT_C, N_OUT], F32, kind="ExternalOutput")

    Exp = mybir.ActivationFunctionType.Exp
    lo_hi = list(windows)
    spans = [0 if lh is None else -(-(lh[1] - lh[0]) // 512) for lh in lo_hi]
    # active n-tiles per m-half, in k order (z accumulation order)
    ks_of = [
        [k for k in range(NT) if lo_hi[k] is not None
         and lo_hi[k][0] < (mh + 1) * 512 and lo_hi[k][1] > mh * 512]
        for mh in range(MH)
    ]
    # dedicated dsq slots when the mask leaves room (needs sum of spans
    # + 2 z banks <= 8), else 2 rotating double-width slots
    dedicated = sum(spans) + 2 <= 8

    with TileContext(nc) as tc:
        with (
            tc.tile_pool(name="const", bufs=1) as cpool,
            tc.tile_pool(name="work", bufs=1) as wpool,
            tc.tile_pool(name="psum", bufs=1, space="PSUM") as ppool,
        ):
            # --- warm exp: trigger the ACT table load at t~0 (no DMA dep) --
            wsrc = cpool.tile([8, 640], BF16)
            nc.vector.memset(wsrc[:], 0.0)
            wact = cpool.tile([8, 16], F32)
            nc.scalar.activation(wact[:], wsrc[:, 0:16], Exp)

            # y^T accumulators: m-half 0 -> col group 0 of bank A,
            # m-half 1 -> col group 1 of bank B (concurrent matmuls AND
            # concurrent DVE/ACT drains)
            zA = ppool.tile([OUT_C, 512], F32, tag="zA", bufs=1)
            zB = ppool.tile([2 * OUT_C, 512], F32, tag="zB", bufs=1)
            zsl = [zA[:], zB[OUT_C:2 * OUT_C, :]]

            # --- input DMAs --------------------------------------------
            # uv gates the d2 pipeline: first on the sync queue.  Row-tiled
            # d2 matmuls need the operands at partition base 32k, so the 8
            # uv rows are replicated to 4 partition offsets (the k=0 copy
            # is the only one on the critical path).
            uvsb = cpool.tile([104, N_IN + N_OUT], BF16)
            nc.sync.dma_start(out=uvsb[0:8, :], in_=uv_d[:])
            nc.gpsimd.dma_start(out=uvsb[32:40, :], in_=uv_d[:])
            stack = cpool.tile([ROWS, N_IN + G * OUT_C], BF16)
            nc.gpsimd.dma_start(out=stack[:], in_=stack_d[:])
            nc.sync.dma_start(out=uvsb[64:72, :], in_=uv_d[:])
            nc.gpsimd.dma_start(out=uvsb[96:104, :], in_=uv_d[:])

            # --- PE warmups: cover the DMA-latency dead zone (they write
            # the unused low partitions of the zB bank)
            for i in range(2):
                nc.tensor.matmul(
                    zB[0:32, :],
                    lhsT=wsrc[:, 0:32],
                    rhs=wsrc[:, 128:640],
                    start=True,
                    stop=True,
                )

            def uL(k):   # (8, 128) d2 lhsT rows for n-tile k, at base 32k
                return uvsb[32 * k:32 * k + 8, k * 128:(k + 1) * 128]

            # --- d2 matmuls (row-tiled) + exp per n-tile -------------------
            # each n-tile only computes its active sorted-m window; one
            # matmul per psum-bank-crossing segment, one exp for the lot
            dsq = []
            for k in range(NT):
                if dedicated:
                    t = ppool.tile([128, (spans[k] or 1) * 512], F32,
                                   tag=f"dsq{k}", bufs=1, name=f"dsq{k}")
                else:
                    t = ppool.tile([128, N_OUT], F32, tag="dsq", bufs=2,
                                   name=f"dsq{k}")
                dsq.append(t)
            esb = {}
            for k in range(NT):
                if lo_hi[k] is None:
                    continue
                lo, hi = lo_hi[k]
                for s0 in range(0, hi - lo, 512):
                    s1 = min(s0 + 512, hi - lo)
                    nc.tensor.matmul(
                        dsq[k][:, s0:s1],
                        lhsT=uL(k),
                        rhs=uvsb[32 * k:32 * k + 8,
                                 N_IN + lo + s0:N_IN + lo + s1],
                        start=True,
                        stop=True,
                        tile_position=(32 * k, 0),
                    )
                for gi in range(G):
                    e = wpool.tile([128, hi - lo], BF16, tag="esb",
                                   bufs=NT if G == 1 else NT + 2,
                                   name=f"e{k}_{gi}")
                    nc.scalar.activation(e[:], dsq[k][:, 0:hi - lo], Exp,
                                         scale=-float(a_tuple[gi]))
                    esb[(k, gi)] = e

            # --- conv matmuls: R2[n, 32g+o] = (r_conv @ lin_w.T)[n, o] ----
            # reuses n-tile 0's dsq bank once its exp has drained
            cps = ppool.tile(
                [128, min(G, 4) * NT * OUT_C], F32,
                tag="dsq0" if dedicated else "dsq",
                bufs=1 if dedicated else 2, name="cps")
            assert G <= 4, "more than 4 length-scale groups unsupported"
            for k in range(NT):
                nc.tensor.matmul(
                    cps[:, k * G * OUT_C:(k + 1) * G * OUT_C],
                    lhsT=stack[:, k * 128:(k + 1) * 128],
                    rhs=stack[0:ROWS, N_IN:N_IN + G * OUT_C],
                    start=True,
                    stop=True,
                )
            rsb = cpool.tile([128, NT * G * OUT_C], BF16)
            nc.vector.tensor_copy(out=rsb[:], in_=cps[:])

            # --- y^T accumulation over active (n-tile, group) -------------
            # ragged column coverage per n-tile is handled by PSUM's
            # per-element has_written bits: the first matmul in a bank
            # clears them, later ones accumulate where set / overwrite
            # where not; every column is covered by >=1 n-tile.
            for k in range(NT):
                if lo_hi[k] is None:
                    continue
                lo, hi = lo_hi[k]
                for gi in range(G):
                    for mh in range(MH):
                        ov0 = max(lo, mh * 512)
                        ov1 = min(hi, (mh + 1) * 512)
                        if ov0 >= ov1:
                            continue
                        nc.tensor.matmul(
                            zsl[mh][:, ov0 - mh * 512:ov1 - mh * 512],
                            lhsT=rsb[:, (k * G + gi) * OUT_C:
                                     (k * G + gi + 1) * OUT_C],
                            rhs=esb[(k, gi)][:, ov0 - lo:ov1 - lo],
                            start=(k == ks_of[mh][0] and gi == 0),
                            stop=(k == ks_of[mh][-1] and gi == G - 1),
                            tile_position=(0, 32 * mh),
                        )

            # --- parallel drains + two contiguous 64KB stores -------------
            osbA = wpool.tile([OUT_C, 512], F32, tag="osbA", bufs=1)
            nc.vector.tensor_copy(out=osbA[:], in_=zsl[0])
            nc.sync.dma_start(out=y_d[:, 0:512], in_=osbA[:])
            # drain and store both on the scalar queue: the DMA issue
            # follows the drain with no cross-engine semaphore hop
            osbB = wpool.tile([OUT_C, 512], F32, tag="osbB", bufs=1)
            nc.scalar.copy(out=osbB[:], in_=zsl[1])
            nc.scalar.dma_start(out=y_d[:, 512:1024], in_=osbB[:])

    _split_multi_waits(nc)
    return nc


_cache = {}


def _get_nc(key):
    if key not in _cache:
        _cache[key] = _build(key)
    return _cache[key]


def _hi_lo(x):
    """Split fp64 array into bf16 hi + bf16 lo with x ~ hi + lo."""
    hi = x.astype(BF)
    lo = (x - hi.astype(np.float64)).astype(BF)
    return hi, lo


def _prepare(r, x_context, y_context, x_target, conv_w, conv_b, sigma, lin_w,
             lin_b):
    r = np.asarray(r, np.float64)
    x_context = np.asarray(x_context, np.float64)
    x_target = np.asarray(x_target, np.float64)
    conv_w = np.asarray(conv_w, np.float64)
    conv_b = np.asarray(conv_b, np.float64)
    sigma = np.asarray(sigma, np.float64)
    lin_w = np.asarray(lin_w, np.float64)
    lin_b = np.asarray(lin_b, np.float64)

    # Channels sharing a length scale share one RBF map: sort channels by a,
    # group runs of equal values (uniform init sigma -> a single group).
    scales = np.exp(sigma)
    a = 0.5 / scales**2
    perm = np.argsort(a, kind="stable")
    a_s = a[perm]
    groups = []
    c0 = 0
    for c in range(1, C + 1):
        if c == C or a_s[c] != a_s[c0]:
            groups.append((c0, c, float(a_s[c0])))
            c0 = c
    a_min = a_s[0]
    G = len(groups)

    # conv weights (channel-permuted), bias row first to pair with the
    # ones row of the im2col stack; FOLD the linear into them: per group
    # W2_g = w_aug[:, c_g] @ lin_w.T[c_g], with lin_b added on the ones
    # row of group 0.
    w_aug = np.concatenate(
        [conv_b[None, :], conv_w.transpose(2, 1, 0).reshape(C * KW, C)], axis=0
    )[:, perm]
    lw = lin_w.T[perm]  # (C, OUT_C), rows in permuted channel order
    w2 = np.zeros((ROWS, G * OUT_C), np.float64)
    for gi, (c0g, c1g, ag) in enumerate(groups):
        w2[:, gi * OUT_C:(gi + 1) * OUT_C] = w_aug[:, c0g:c1g] @ lw[c0g:c1g]
    w2[0, 0:OUT_C] += lin_b
    w2 = w2.astype(BF)

    pad = KW // 2
    gapmax = float(np.sqrt(BAND_T / a_min))
    in_maps = []
    w_lo = [N_OUT] * NT
    w_hi = [0] * NT
    gathers = []
    for b in range(B):
        u_raw = x_context[b, :, 0]
        v_raw = x_target[b, :, 0]
        u_idx = np.argsort(u_raw, kind="stable")
        v_idx = np.argsort(v_raw, kind="stable")
        u = u_raw[u_idx]
        v = v_raw[v_idx]

        # banding: n-tile k only interacts with sorted-m ranks whose xt
        # lies within gapmax of the tile's xc range (union over batches,
        # rounded to 16-col boundaries)
        for k in range(NT):
            ulo, uhi = u[k * 128], u[(k + 1) * 128 - 1]
            r0 = int(np.searchsorted(v, ulo - gapmax, side="left"))
            r1 = int(np.searchsorted(v, uhi + gapmax, side="right"))
            w_lo[k] = min(w_lo[k], (r0 // 16) * 16)
            w_hi[k] = max(w_hi[k], -(-r1 // 16) * 16)

        # host im2col: ones row + 5 shifted copies of r (pure layout),
        # then permute columns into sorted-xc order; folded weights ride
        # along in the trailing columns (one DMA).
        stack = np.zeros((ROWS, N_IN), np.float64)
        stack[0] = 1.0
        rb = r[b]
        for k in range(KW):
            lo = max(0, pad - k)
            hi = min(N_IN, N_IN + pad - k)
            stack[1 + C * k:1 + C * (k + 1), lo:hi] = rb[:, lo + k - pad:hi + k - pad]
        stack = np.concatenate([stack[:, u_idx].astype(BF), w2], axis=1)

        # d2 factor rows: d2 = u^2 - 2uv + v^2 with exact bf16 products
        uh, ul = _hi_lo(u)
        vh, vl = _hi_lo(v)
        suh, sul = _hi_lo(u * u)
        svh, svl = _hi_lo(v * v)
        one_n = np.ones(N_IN, BF)
        one_m = np.ones(N_OUT, BF)
        zero_n = np.zeros(N_IN, BF)
        zero_m = np.zeros(N_OUT, BF)
        uL = np.stack([suh, sul,
                       (-2.0 * uh.astype(np.float64)).astype(BF),
                       (-2.0 * ul.astype(np.float64)).astype(BF),
                       (-2.0 * uh.astype(np.float64)).astype(BF),
                       one_n, one_n, zero_n])
        vR = np.stack([one_m, one_m, vh, vh, vl, svh, svl, zero_m])
        uv = np.concatenate([uL, vR], axis=1)

        in_maps.append({
            "stack": np.ascontiguousarray(stack),
            "uv": np.ascontiguousarray(uv),
        })

        # device column = sorted rank; host maps back to original order
        inv = np.empty(N_OUT, np.int64)
        inv[v_idx] = np.arange(N_OUT)
        gathers.append(inv)

    windows = tuple(
        (w_lo[k], min(w_hi[k], N_OUT)) if w_hi[k] > w_lo[k] else None
        for k in range(NT)
    )
    key = (G, tuple(float(g[2]) for g in groups), windows)
    return key, in_maps, gathers


def _assemble(res, gathers):
    return np.stack(
        [res.results[b]["y"].T[gathers[b]] for b in range(B)], axis=0
    )


def kernel(**inputs):
    key, in_maps, gathers = _prepare(**inputs)
    nc = _get_nc(key)
    res = run_bass_kernel_spmd(nc, in_maps, list(range(N_CORES)))
    return _assemble(res, gathers)


# revision 30
# speedup vs baseline: 1.0438x; 1.0438x over previous
"""ConvDecoder Bass kernel for Trainium2, SPMD over 8 NeuronCores.

Math (per batch element b, one per core):
    r_conv = Conv1d(r, conv_w, SAME) + conv_b            # (C, N_IN)
    d[n,m] = (xc[n] - xt[m])^2                           # (N_IN, N_OUT)
    E_c    = exp(-a_c * d),  a_c = 0.5 / exp(sigma_c)^2
    z[m,c] = sum_n r_conv[c,n] * E_c[n,m]
    out    = z @ lin_w.T + lin_b                         # (N_OUT, OUT_C)

Per-core structure (v7):
  - The pointwise linear is FOLDED INTO THE CONV WEIGHTS on host:
    W2_g = w_aug[:, c_g] @ lin_w.T[c_g] (+ lin_b on the im2col ones
    row), one block per length-scale group g.  The conv matmul then
    yields R2[n, o] = sum_c r_conv[n, c] lin_w[o, c] directly, and
    the z matmul produces y^T = sum_n R2[n,:]^T E[n,:] -- the output
    itself in (OUT_C, m) layout.  No on-device linear stage at all;
    the host transposes (free) when assembling.
  - im2col stack built on HOST in sorted-xc order; W2 rides in its
    trailing columns (ONE input DMA for all weights/data except uv).
  - d^2 on the PE as a rank-7 bf16 matmul (exact hi/lo splits), one
    row-tile position per n-tile so the matmuls overlap; ACT exp
    reads PSUM, writes bf16 E to SBUF.  ACT is the serial bottleneck.
  - BANDING: xc/xt sorted on host => far (n-tile, m-half) blocks are
    skipped (mask from the actual data, union over batches, baked
    into the compiled kernel).  With the usual 6/8 mask each dsq
    n-tile gets a DEDICATED PSUM slot (no rotation, d^2 never waits
    on ACT); the conv shares n-tile 0's bank after its exp drains.
  - y^T accumulates in two PSUM banks (m-half 0 -> partitions 0:32
    col group 0, m-half 1 -> 32:64 col group 1, concurrent), drained
    in parallel by DVE/ACT, stored by two contiguous 64KB DMAs in
    sorted order; host inverse-gathers rows.
"""

import numpy as np
import ml_dtypes

import concourse.bass as bass
import concourse.mybir as mybir
from concourse.tile import TileContext
from concourse.bass_utils import run_bass_kernel_spmd

F32 = mybir.dt.float32
BF16 = mybir.dt.bfloat16
BF = ml_dtypes.bfloat16

B, N_IN, N_OUT, C, OUT_C, KW = 8, 512, 1024, 16, 32, 5
N_CORES = 8
NT = N_IN // 128   # n tiles (4)
MH = N_OUT // 512  # m halves (2)
ROWS = C * KW + 1  # im2col rows (81)
BAND_T = 16.0      # skip a block when a * gap^2 > BAND_T (exp < 1.2e-7)


# --- walrus workaround -----------------------------------------------------
# This container's walrus accepts at most ONE semaphore wait per TPB
# instruction, but Tile's scheduler attaches several (joins + tail drain).
# Hoist all but the last wait of each instruction onto fresh wait-only
# EventSemaphore instructions inserted right before it on the same engine.
_ws_ctr = [0]


def _split_multi_waits(nc):
    for fn in nc.m.functions:
        for blk in fn.blocks:
            insts = blk.instructions
            if not any(
                ins.sync_info and len(ins.sync_info.on_wait) > 1 for ins in insts
            ):
                continue
            out = []
            for ins in insts:
                si = ins.sync_info
                waits = list(si.on_wait) if si else []
                if len(waits) > 1:
                    for w in waits[:-1]:
                        _ws_ctr[0] += 1
                        ev = mybir.InstEventSemaphore(
                            name=f"waitsplit_{_ws_ctr[0]}", ins=[], outs=[]
                        )
                        ev.engine = ins.engine
                        ev.sync_info = mybir.SyncInfo(on_wait=[w], on_update=[])
                        nc.register_instruction(ev)
                        out.append(ev)
                    ins.sync_info = mybir.SyncInfo(
                        on_wait=[waits[-1]], on_update=list(si.on_update)
                    )
                out.append(ins)
            insts[:] = out


# --- kernel build ----------------------------------------------------------
def _build(key):
    """key: (n_groups, a_tuple, windows) - a_tuple the per-group exp
    scales, windows the per-n-tile active sorted-m column range."""
    n_groups, a_tuple, windows = key
    G = n_groups
    nc = bass.Bass()
    # stack carries the im2col block (cols 0:512) and the folded
    # conv+linear weights W2 per group (cols 512:512+32G)
    stack_d = nc.dram_tensor("stack", [ROWS, N_IN + G * OUT_C], BF16,
                             kind="ExternalInput")
    uv_d = nc.dram_tensor("uv", [8, N_IN + N_OUT], BF16, kind="ExternalInput")
    # output is y^T in sorted-m order; host transposes + inverse-gathers
    y_d = nc.dram_tensor("y", [OUT_C, N_OUT], F32, kind="ExternalOutput")

    Exp = mybir.ActivationFunctionType.Exp
    lo_hi = list(windows)
    spans = [0 if lh is None else -(-(lh[1] - lh[0]) // 512) for lh in lo_hi]
    # active n-tiles per m-half, in k order (z accumulation order)
    ks_of = [
        [k for k in range(NT) if lo_hi[k] is not None
         and lo_hi[k][0] < (mh + 1) * 512 and lo_hi[k][1] > mh * 512]
        for mh in range(MH)
    ]
    # dedicated dsq slots when the mask leaves room (needs sum of spans
    # + 2 z banks <= 8), else 2 rotating double-width slots
    dedicated = sum(spans) + 2 <= 8

    with TileContext(nc) as tc:
        with (
            tc.tile_pool(name="const", bufs=1) as cpool,
            tc.tile_pool(name="work", bufs=1) as wpool,
            tc.tile_pool(name="psum", bufs=1, space="PSUM") as ppool,
        ):
            # --- warm exp: trigger the ACT table load at t~0 (no DMA dep) --
            wsrc = cpool.tile([8, 640], BF16)
            nc.vector.memset(wsrc[:], 0.0)
            wact = cpool.tile([8, 16], F32)
            nc.scalar.activation(wact[:], wsrc[:, 0:16], Exp)

            # y^T accumulators: m-half 0 -> col group 0 of bank A,
            # m-half 1 -> col group 1 of bank B (concurrent matmuls AND
            # concurrent DVE/ACT drains)
            zA = ppool.tile([OUT_C, 512], F32, tag="zA", bufs=1)
            zB = ppool.tile([2 * OUT_C, 512], F32, tag="zB", bufs=1)
            zsl = [zA[:], zB[OUT_C:2 * OUT_C, :]]

            # --- input DMAs --------------------------------------------
            # uv gates the d2 pipeline: first on the sync queue.  Row-tiled
            # d2 matmuls need the operands at partition base 32k, so the 8
            # uv rows are replicated to 4 partition offsets (the k=0 copy
            # is the only one on the critical path).
            uvsb = cpool.tile([104, N_IN + N_OUT], BF16)
            nc.sync.dma_start(out=uvsb[0:8, :], in_=uv_d[:])
            nc.gpsimd.dma_start(out=uvsb[32:40, :], in_=uv_d[:])
            stack = cpool.tile([ROWS, N_IN + G * OUT_C], BF16)
            nc.gpsimd.dma_start(out=stack[:], in_=stack_d[:])
            nc.sync.dma_start(out=uvsb[64:72, :], in_=uv_d[:])
            # the 4th replica issues from the scalar queue (right after
            # the table-load + warm exp) so no queue is 3 issues deep
            nc.scalar.dma_start(out=uvsb[96:104, :], in_=uv_d[:])

            # --- PE warmups: cover the DMA-latency dead zone (they write
            # the unused low partitions of the zB bank)
            for i in range(2):
                nc.tensor.matmul(
                    zB[0:32, :],
                    lhsT=wsrc[:, 0:32],
                    rhs=wsrc[:, 128:640],
                    start=True,
                    stop=True,
                )

            def uL(k):   # (8, 128) d2 lhsT rows for n-tile k, at base 32k
                return uvsb[32 * k:32 * k + 8, k * 128:(k + 1) * 128]

            # --- d2 matmuls (row-tiled) + exp per n-tile -------------------
            # each n-tile only computes its active sorted-m window; one
            # matmul per psum-bank-crossing segment, one exp for the lot
            dsq = []
            for k in range(NT):
                if dedicated:
                    t = ppool.tile([128, (spans[k] or 1) * 512], F32,
                                   tag=f"dsq{k}", bufs=1, name=f"dsq{k}")
                else:
                    t = ppool.tile([128, N_OUT], F32, tag="dsq", bufs=2,
                                   name=f"dsq{k}")
                dsq.append(t)
            esb = {}
            for k in range(NT):
                if lo_hi[k] is None:
                    continue
                lo, hi = lo_hi[k]
                for s0 in range(0, hi - lo, 512):
                    s1 = min(s0 + 512, hi - lo)
                    nc.tensor.matmul(
                        dsq[k][:, s0:s1],
                        lhsT=uL(k),
                        rhs=uvsb[32 * k:32 * k + 8,
                                 N_IN + lo + s0:N_IN + lo + s1],
                        start=True,
                        stop=True,
                        tile_position=(32 * k, 0),
                    )
                for gi in range(G):
                    e = wpool.tile([128, hi - lo], BF16, tag="esb",
                                   bufs=NT if G == 1 else NT + 2,
                                   name=f"e{k}_{gi}")
                    nc.scalar.activation(e[:], dsq[k][:, 0:hi - lo], Exp,
                                         scale=-float(a_tuple[gi]))
                    esb[(k, gi)] = e

            # --- conv matmuls: R2[n, 32g+o] = (r_conv @ lin_w.T)[n, o] ----
            # reuses n-tile 0's dsq bank once its exp has drained
            cps = ppool.tile(
                [128, min(G, 4) * NT * OUT_C], F32,
                tag="dsq0" if dedicated else "dsq",
                bufs=1 if dedicated else 2, name="cps")
            assert G <= 4, "more than 4 length-scale groups unsupported"
            for k in range(NT):
                nc.tensor.matmul(
                    cps[:, k * G * OUT_C:(k + 1) * G * OUT_C],
                    lhsT=stack[:, k * 128:(k + 1) * 128],
                    rhs=stack[0:ROWS, N_IN:N_IN + G * OUT_C],
                    start=True,
                    stop=True,
                )
            rsb = cpool.tile([128, NT * G * OUT_C], BF16)
            nc.vector.tensor_copy(out=rsb[:], in_=cps[:])

            # --- y^T accumulation over active (n-tile, group) -------------
            # ragged column coverage per n-tile is handled by PSUM's
            # per-element has_written bits: the first matmul in a bank
            # clears them, later ones accumulate where set / overwrite
            # where not; every column is covered by >=1 n-tile.
            for k in range(NT):
                if lo_hi[k] is None:
                    continue
                lo, hi = lo_hi[k]
                for gi in range(G):
                    for mh in range(MH):
                        ov0 = max(lo, mh * 512)
                        ov1 = min(hi, (mh + 1) * 512)
                        if ov0 >= ov1:
                            continue
                        nc.tensor.matmul(
                            zsl[mh][:, ov0 - mh * 512:ov1 - mh * 512],
                            lhsT=rsb[:, (k * G + gi) * OUT_C:
                                     (k * G + gi + 1) * OUT_C],
                            rhs=esb[(k, gi)][:, ov0 - lo:ov1 - lo],
                            start=(k == ks_of[mh][0] and gi == 0),
                            stop=(k == ks_of[mh][-1] and gi == G - 1),
                            tile_position=(0, 32 * mh),
                        )

            # --- parallel drains + two contiguous 64KB stores -------------
            osbA = wpool.tile([OUT_C, 512], F32, tag="osbA", bufs=1)
            nc.vector.tensor_copy(out=osbA[:], in_=zsl[0])
            nc.sync.dma_start(out=y_d[:, 0:512], in_=osbA[:])
            # drain and store both on the scalar queue: the DMA issue
            # follows the drain with no cross-engine semaphore hop
            osbB = wpool.tile([OUT_C, 512], F32, tag="osbB", bufs=1)
            nc.scalar.copy(out=osbB[:], in_=zsl[1])
            nc.scalar.dma_start(out=y_d[:, 512:1024], in_=osbB[:])

    _split_multi_waits(nc)
    return nc


_cache = {}


def _get_nc(key):
    if key not in _cache:
        _cache[key] = _build(key)
    return _cache[key]


def _hi_lo(x):
    """Split fp64 array into bf16 hi + bf16 lo with x ~ hi + lo."""
    hi = x.astype(BF)
    lo = (x - hi.astype(np.float64)).astype(BF)
    return hi, lo


def _prepare(r, x_context, y_context, x_target, conv_w, conv_b, sigma, lin_w,
             lin_b):
    r = np.asarray(r, np.float64)
    x_context = np.asarray(x_context, np.float64)
    x_target = np.asarray(x_target, np.float64)
    conv_w = np.asarray(conv_w, np.float64)
    conv_b = np.asarray(conv_b, np.float64)
    sigma = np.asarray(sigma, np.float64)
    lin_w = np.asarray(lin_w, np.float64)
    lin_b = np.asarray(lin_b, np.float64)

    # Channels sharing a length scale share one RBF map: sort channels by a,
    # group runs of equal values (uniform init sigma -> a single group).
    scales = np.exp(sigma)
    a = 0.5 / scales**2
    perm = np.argsort(a, kind="stable")
    a_s = a[perm]
    groups = []
    c0 = 0
    for c in range(1, C + 1):
        if c == C or a_s[c] != a_s[c0]:
            groups.append((c0, c, float(a_s[c0])))
            c0 = c
    a_min = a_s[0]
    G = len(groups)

    # conv weights (channel-permuted), bias row first to pair with the
    # ones row of the im2col stack; FOLD the linear into them: per group
    # W2_g = w_aug[:, c_g] @ lin_w.T[c_g], with lin_b added on the ones
    # row of group 0.
    w_aug = np.concatenate(
        [conv_b[None, :], conv_w.transpose(2, 1, 0).reshape(C * KW, C)], axis=0
    )[:, perm]
    lw = lin_w.T[perm]  # (C, OUT_C), rows in permuted channel order
    w2 = np.zeros((ROWS, G * OUT_C), np.float64)
    for gi, (c0g, c1g, ag) in enumerate(groups):
        w2[:, gi * OUT_C:(gi + 1) * OUT_C] = w_aug[:, c0g:c1g] @ lw[c0g:c1g]
    w2[0, 0:OUT_C] += lin_b
    w2 = w2.astype(BF)

    pad = KW // 2
    gapmax = float(np.sqrt(BAND_T / a_min))
    in_maps = []
    w_lo = [N_OUT] * NT
    w_hi = [0] * NT
    gathers = []
    for b in range(B):
        u_raw = x_context[b, :, 0]
        v_raw = x_target[b, :, 0]
        u_idx = np.argsort(u_raw, kind="stable")
        v_idx = np.argsort(v_raw, kind="stable")
        u = u_raw[u_idx]
        v = v_raw[v_idx]

        # banding: n-tile k only interacts with sorted-m ranks whose xt
        # lies within gapmax of the tile's xc range (union over batches,
        # rounded to 16-col boundaries)
        for k in range(NT):
            ulo, uhi = u[k * 128], u[(k + 1) * 128 - 1]
            r0 = int(np.searchsorted(v, ulo - gapmax, side="left"))
            r1 = int(np.searchsorted(v, uhi + gapmax, side="right"))
            w_lo[k] = min(w_lo[k], (r0 // 16) * 16)
            w_hi[k] = max(w_hi[k], -(-r1 // 16) * 16)

        # host im2col: ones row + 5 shifted copies of r (pure layout),
        # then permute columns into sorted-xc order; folded weights ride
        # along in the trailing columns (one DMA).
        stack = np.zeros((ROWS, N_IN), np.float64)
        stack[0] = 1.0
        rb = r[b]
        for k in range(KW):
            lo = max(0, pad - k)
            hi = min(N_IN, N_IN + pad - k)
            stack[1 + C * k:1 + C * (k + 1), lo:hi] = rb[:, lo + k - pad:hi + k - pad]
        stack = np.concatenate([stack[:, u_idx].astype(BF), w2], axis=1)

        # d2 factor rows: d2 = u^2 - 2uv + v^2 with exact bf16 products
        uh, ul = _hi_lo(u)
        vh, vl = _hi_lo(v)
        suh, sul = _hi_lo(u * u)
        svh, svl = _hi_lo(v * v)
        one_n = np.ones(N_IN, BF)
        one_m = np.ones(N_OUT, BF)
        zero_n = np.zeros(N_IN, BF)
        zero_m = np.zeros(N_OUT, BF)
        uL = np.stack([suh, sul,
                       (-2.0 * uh.astype(np.float64)).astype(BF),
                       (-2.0 * ul.astype(np.float64)).astype(BF),
                       (-2.0 * uh.astype(np.float64)).astype(BF),
                       one_n, one_n, zero_n])
        vR = np.stack([one_m, one_m, vh, vh, vl, svh, svl, zero_m])
        uv = np.concatenate([uL, vR], axis=1)

        in_maps.append({
            "stack": np.ascontiguousarray(stack),
            "uv": np.ascontiguousarray(uv),
        })

        # device column = sorted rank; host maps back to original order
        inv = np.empty(N_OUT, np.int64)
        inv[v_idx] = np.arange(N_OUT)
        gathers.append(inv)

    windows = tuple(
        (w_lo[k], min(w_hi[k], N_OUT)) if w_hi[k] > w_lo[k] else None
        for k in range(NT)
    )
    key = (G, tuple(float(g[2]) for g in groups), windows)
    return key, in_maps, gathers


def _assemble(res, gathers):
    return np.stack(
        [res.results[b]["y"].T[gathers[b]] for b in range(B)], axis=0
    )


def kernel(**inputs):
    key, in_maps, gathers = _prepare(**inputs)
    nc = _get_nc(key)
    res = run_bass_kernel_spmd(nc, in_maps, list(range(N_CORES)))
    return _assemble(res, gathers)
